# revision 12
# baseline (speedup 1.0000x reference)
"""Multi-sense skip-gram (MSSG) loss kernel for Trainium2 — v2.

Data-parallel over batch across 8 NeuronCores; tables replicated in bf16.

v2 replaces v1's 64 serialized one-index-per-partition indirect DMAs per
core with 16 vectorized SWDGE dma_gather calls (InstDMAGatherAnt, mlp Q7
library): each call gathers hundreds of rows in one instruction, with
descriptors generated at ~0.34ns each and sprayed across all 16 DMA
engines.

dma_gather indices are int16 (sign-extended by the ucode), so vocab ids
up to 49999 cannot be used directly. Pairing trick: the packed table
([50000, 2176] bf16 rows: [glob 300 | emb 900 | dis 900 | pad]) is viewed
as [25000, 4352] two-row windows; idx = id>>1 <= 24999 fits int16. The
correct half is selected on-device: sel = A*(1-par) + B*par, exact since
par is 0.0/1.0, with both parity and complement supplied as host-packed
[P,1] f32 columns. The A-half multiply runs as an ACT-engine Copy-with-
scale (ACT is otherwise idle), the B-half fused multiply-add on DVE.
Costs 2x gather bytes vs. the ideal but is ~100x less per-descriptor
overhead than the v1 path. Negative samples only need the global
embedding, so they gather 1280B pair windows from a separate compact
[25000, 640] glob table instead of full 8704B row pairs.

Compute (per 128-element tile, one element per SBUF partition) is
unchanged from v1: DVE does batched dots/weighted sums along the free
dim, ACT does exp/ln, final partition-reduce via ones-vector matmul on
PE. Host sums the 8 per-core partials.
"""

import numpy as np

NUM_SENSE = 3
EMB_DIM = 300
VOCAB = 50000
BATCH = 4096
CTX = 10
NEG = 5
N_CORES = 8
P = 128
PER_CORE = BATCH // N_CORES  # 512
TILES = PER_CORE // P        # 4
D = EMB_DIM
CS = CTX * NUM_SENSE         # 30
SN = NUM_SENSE * NEG         # 15

ROW = 2176                   # bf16 elems per packed row (4352B, 17*256)
WIN = 2 * ROW                # pair window
NEG_W = 320                  # neg table: per-row glob slot (640B)
NEG_ELEM = 2 * NEG_W         # 640 elems = 1280B pair window
EMB_OFF = D                  # 300
DIS_OFF = D + NUM_SENSE * D  # 1200
SEL_W = DIS_OFF + NUM_SENSE * D  # 2100 used elems per row

# idx columns per tile: ctxA 40 | ctxB 40 | neg 40 | word 8 = 128
IDX_COLS = 128
PAR_COLS = 32                # [par: ctx 10|neg 5|word 1] + [1-par x16]

_CACHE = {}


def _build_bass(tiles=TILES, passes=1):
    key = ("v2", tiles, passes)
    if key in _CACHE:
        return _CACHE[key]

    import concourse.bass as bass
    import concourse.bacc as bacc
    import concourse.tile as tile
    from concourse import mybir
    from concourse.library_config import mlp

    F32 = mybir.dt.float32
    BF16 = mybir.dt.bfloat16
    I16 = mybir.dt.int16
    AX = mybir.AxisListType
    OP = mybir.AluOpType
    AF = mybir.ActivationFunctionType
    TINY = float(np.finfo(np.float32).tiny)

    nc = bacc.Bacc("TRN2", target_bir_lowering=False, debug=False)

    packed = nc.dram_tensor("packed", [VOCAB // 2, WIN], BF16,
                            kind="ExternalInput")
    negtab = nc.dram_tensor("negtab", [VOCAB // 2, NEG_ELEM], BF16,
                            kind="ExternalInput")
    idx = nc.dram_tensor("idx", [P, tiles * IDX_COLS], I16,
                         kind="ExternalInput")
    par = nc.dram_tensor("par", [P, tiles * PAR_COLS], F32,
                         kind="ExternalInput")
    out_d = nc.dram_tensor("out", [1, 1], F32, kind="ExternalOutput")

    def tt(out, a, b, op=OP.add):
        nc.vector.tensor_tensor(out=out, in0=a, in1=b, op=op)

    with tile.TileContext(nc) as tc:
        with (
            tc.tile_pool(name="gw", bufs=2) as gw,
            tc.tile_pool(name="sel", bufs=1) as slp,
            tc.tile_pool(name="tmpp", bufs=1) as tp,
            tc.tile_pool(name="small", bufs=2) as sp,
            tc.tile_pool(name="persist", bufs=1) as pp,
            tc.tile_pool(name="psum", bufs=1, space="PSUM") as psp,
        ):
            nc.gpsimd.load_library(mlp)

            acc = pp.tile([P, 2 * TILES], F32)
            ones = pp.tile([P, 1], F32)
            nc.vector.memset(ones[:], 1.0)
            IX = pp.tile([P, tiles * IDX_COLS], I16)
            PAR = pp.tile([P, tiles * PAR_COLS], F32)
            nc.sync.dma_start(out=IX[:], in_=idx[:, :])
            nc.sync.dma_start(out=PAR[:], in_=par[:, :])

            SELC = slp.tile([P, CTX * ROW], BF16)   # selected ctx rows
            SELN = slp.tile([P, NEG * 320], BF16)   # selected neg glob
            SELW = slp.tile([P, 2 * NUM_SENSE * D], BF16)  # word emb|dis

            for t_iter in range(tiles * passes):
                t = t_iter % tiles
                ib = t * IDX_COLS
                pb = t * PAR_COLS

                def select(dst, a, b, par_col):
                    # dst = a*(1-par) + b*par  (exact: par is 0.0 or 1.0)
                    # a*(1-par) runs on the otherwise-idle ACT engine
                    omp_col = pb + PAR_COLS // 2 + (par_col - pb)
                    nc.scalar.activation(
                        out=dst, in_=a, func=AF.Copy,
                        scale=PAR[:, omp_col:omp_col + 1])
                    nc.vector.scalar_tensor_tensor(
                        out=dst, in0=b, scalar=PAR[:, par_col:par_col + 1],
                        in1=dst, op0=OP.mult, op1=OP.add)

                # ctx: 2 gathers x 640 windows
                for h in range(2):
                    W = gw.tile([P, 5 * WIN], BF16, tag="wc")
                    W3 = W[:].rearrange("p (c w) -> p c w", w=WIN)
                    nc.gpsimd.dma_gather(
                        W3, packed[:, :],
                        IX[:, ib + h * 40:ib + (h + 1) * 40],
                        640, 640, WIN)
                    for c in range(5):
                        cc = h * 5 + c
                        select(SELC[:, cc * ROW:cc * ROW + SEL_W],
                               W3[:, c, 0:SEL_W], W3[:, c, ROW:ROW + SEL_W],
                               pb + cc)

                # neg: 640 pair windows from the compact glob table
                WN = gw.tile([P, 5 * NEG_ELEM], BF16, tag="wn")
                WN3 = WN[:].rearrange("p (c w) -> p c w", w=NEG_ELEM)
                nc.gpsimd.dma_gather(
                    WN3, negtab[:, :],
                    IX[:, ib + 80:ib + 120],
                    640, 640, NEG_ELEM)
                for n in range(5):
                    select(SELN[:, n * 320:n * 320 + 320],
                           WN3[:, n, 0:NEG_W], WN3[:, n, NEG_W:2 * NEG_W],
                           pb + 10 + n)

                # word: 128 windows; only emb|dis sections selected
                WW = gw.tile([P, WIN], BF16, tag="ww")
                WW3 = WW[:].rearrange("p (c w) -> p c w", w=WIN)
                nc.gpsimd.dma_gather(
                    WW3, packed[:, :],
                    IX[:, ib + 120:ib + 128],
                    128, 128, WIN)
                select(SELW[:],
                       WW3[:, 0, EMB_OFF:SEL_W],
                       WW3[:, 0, ROW + EMB_OFF:ROW + SEL_W],
                       pb + 15)

                # ---- views over selected rows ----
                SEL3 = SELC[:].rearrange("p (c x) -> p c x", x=ROW)
                CT3 = SEL3[:, :, 0:D]                                  # [P,10,300]
                AS4 = SEL3[:, :, EMB_OFF:DIS_OFF].rearrange(
                    "p c (s d) -> p c s d", d=D)                       # [P,10,3,300]
                AD4 = SEL3[:, :, DIS_OFF:SEL_W].rearrange(
                    "p c (s d) -> p c s d", d=D)
                SEN3 = SELW[:, 0:NUM_SENSE * D].rearrange(
                    "p (s d) -> p s d", d=D)                           # [P,3,300]
                DIS3 = SELW[:, NUM_SENSE * D:2 * NUM_SENSE * D].rearrange(
                    "p (s d) -> p s d", d=D)
                NG3 = SELN[:].rearrange("p (n x) -> p n x", x=320)[:, :, 0:D]

                def as_seg(k):
                    c, s = k // NUM_SENSE, k % NUM_SENSE
                    return AS4[:, c, s, :]

                TMP = tp.tile([P, CS * D], BF16)

                def seg3(ap):
                    return ap.rearrange("p (c d) -> p c d", d=D)

                def dot_segments(a_full, b_full, out_full, zout, nseg):
                    """zout[:, k] = sum_d a_seg(k) * b_seg(k) (dots of 300-vecs)
                    via elementwise mult into TMP + fold tree + short reduce
                    (DVE tensor_reduce runs at ~1 elem/cycle vs 2x for TT, so
                    folds first, reduce only the last 38)."""
                    tt(out_full, a_full, b_full, OP.mult)
                    pr = seg3(TMP[:])[:, 0:nseg, :]
                    tt(pr[:, :, 0:150], pr[:, :, 0:150], pr[:, :, 150:300])
                    tt(pr[:, :, 0:74], pr[:, :, 0:74], pr[:, :, 76:150])
                    tt(pr[:, :, 0:38], pr[:, :, 0:38], pr[:, :, 38:76])
                    nc.vector.tensor_reduce(
                        out=zout, in_=pr[:, :, 0:38], axis=AX.X, op=OP.add)

                # ---- ctx1 = sum_c CT (mean deferred via exp-scale) ----
                # fold tree through TMP slices (TMP is free at this point)
                c1a = TMP[:, 0:5 * D]
                c1b = TMP[:, 5 * D:7 * D]
                ctx1 = sp.tile([P, D], BF16, tag="ctx1")
                tt(seg3(c1a), CT3[:, 0:5, :], CT3[:, 5:10, :])
                tt(c1b, c1a[:, 0:2 * D], c1a[:, 2 * D:4 * D])
                tt(c1b[:, 0:D], c1b[:, 0:D], c1b[:, D:2 * D])
                tt(ctx1[:], c1b[:, 0:D], c1a[:, 4 * D:5 * D])

                def disamb_step(ctx_vec, ctx_out):
                    z = sp.tile([P, CS], F32, tag="z")
                    dot_segments(
                        AD4,
                        ctx_vec[:].unsqueeze(1).unsqueeze(1)
                                  .to_broadcast([P, CTX, NUM_SENSE, D]),
                        TMP[:].rearrange("p (c s d) -> p c s d",
                                         s=NUM_SENSE, d=D),
                        z[:], CS)
                    E = sp.tile([P, CS], F32, tag="E")
                    nc.scalar.activation(out=E[:], in_=z[:], func=AF.Exp,
                                         scale=1.0 / CTX)
                    S = sp.tile([P, CTX], F32, tag="S")
                    nc.vector.tensor_reduce(
                        out=S[:], in_=E[:].rearrange("p (c s) -> p c s",
                                                     s=NUM_SENSE),
                        axis=AX.X, op=OP.add)
                    R = sp.tile([P, CTX], F32, tag="R")
                    nc.vector.reciprocal(R[:], S[:])
                    AL = sp.tile([P, CS], F32, tag="AL")
                    tt(AL[:].rearrange("p (c s) -> p c s", s=NUM_SENSE),
                       E[:].rearrange("p (c s) -> p c s", s=NUM_SENSE),
                       R[:].unsqueeze(2).to_broadcast([P, CTX, NUM_SENSE]),
                       OP.mult)
                    T3 = seg3(TMP[:])
                    for cs in range(CS):
                        nc.vector.tensor_scalar_mul(
                            out=T3[:, cs, :], in0=as_seg(cs),
                            scalar1=AL[:, cs:cs + 1])
                    tt(T3[:, 0:15, :], T3[:, 0:15, :], T3[:, 15:30, :])
                    tt(T3[:, 0:7, :], T3[:, 0:7, :], T3[:, 7:14, :])
                    tt(T3[:, 0:3, :], T3[:, 0:3, :], T3[:, 3:6, :])
                    tt(T3[:, 0:1, :], T3[:, 0:1, :], T3[:, 1:2, :])
                    tt(T3[:, 0:1, :], T3[:, 0:1, :], T3[:, 14:15, :])
                    tt(T3[:, 2:3, :], T3[:, 2:3, :], T3[:, 6:7, :])
                    tt(ctx_out[:].unsqueeze(1), T3[:, 0:1, :], T3[:, 2:3, :])

                ctx2 = sp.tile([P, D], BF16, tag="ctx2")
                ctx3 = sp.tile([P, D], BF16, tag="ctx3")
                disamb_step(ctx1, ctx2)
                disamb_step(ctx2, ctx3)

                # ---- alpha = softmax_s(DIS . ctx3/C) ----
                av = sp.tile([P, NUM_SENSE], F32, tag="av")
                dot_segments(
                    DIS3,
                    ctx3[:].unsqueeze(1).to_broadcast([P, NUM_SENSE, D]),
                    seg3(TMP[:])[:, 0:NUM_SENSE, :], av[:], NUM_SENSE)
                Ea = sp.tile([P, NUM_SENSE], F32, tag="Ea")
                nc.scalar.activation(out=Ea[:], in_=av[:], func=AF.Exp,
                                     scale=1.0 / CTX)
                Sa = sp.tile([P, 1], F32, tag="Sa")
                nc.vector.tensor_reduce(out=Sa[:], in_=Ea[:], axis=AX.X,
                                        op=OP.add)
                Ra = sp.tile([P, 1], F32, tag="Ra")
                nc.vector.reciprocal(Ra[:], Sa[:])
                ALS = sp.tile([P, NUM_SENSE], F32, tag="ALS")
                nc.vector.tensor_scalar_mul(out=ALS[:], in0=Ea[:],
                                            scalar1=Ra[:, 0:1])

                # ---- pos: sigmoid(SEN . CT) over (s,c) ----
                PL = sp.tile([P, CS], F32, tag="PL")
                dot_segments(
                    CT3.unsqueeze(1).to_broadcast([P, NUM_SENSE, CTX, D]),
                    SEN3.unsqueeze(2).to_broadcast([P, NUM_SENSE, CTX, D]),
                    TMP[:].rearrange("p (s c d) -> p s c d",
                                     s=NUM_SENSE, c=CTX),
                    PL[:], CS)
                EP = sp.tile([P, CS], F32, tag="EP")
                nc.scalar.activation(out=EP[:], in_=PL[:], func=AF.Exp,
                                     scale=-1.0)
                nc.vector.tensor_scalar_add(out=EP[:], in0=EP[:], scalar1=1.0)
                PP = sp.tile([P, CS], F32, tag="PP")
                nc.vector.reciprocal(PP[:], EP[:])

                # pos_term[c] = sum_s alpha_s * PP[s,c]
                W_ = sp.tile([P, CS], F32, tag="W")
                for s in range(NUM_SENSE):
                    nc.vector.tensor_scalar_mul(
                        out=W_[:, s * CTX:(s + 1) * CTX],
                        in0=PP[:, s * CTX:(s + 1) * CTX],
                        scalar1=ALS[:, s:s + 1])
                tt(W_[:, 0:CTX], W_[:, 0:CTX], W_[:, CTX:2 * CTX])
                tt(W_[:, 0:CTX], W_[:, 0:CTX], W_[:, 2 * CTX:3 * CTX])
                nc.vector.tensor_scalar_max(out=W_[:, 0:CTX], in0=W_[:, 0:CTX],
                                            scalar1=TINY)
                WL = sp.tile([P, CTX], F32, tag="WL")
                nc.scalar.activation(out=WL[:], in_=W_[:, 0:CTX], func=AF.Ln,
                                     accum_out=acc[:, 2 * t:2 * t + 1])

                # ---- neg: sigmoid(SEN . NG) over (s,n) ----
                NL = sp.tile([P, SN], F32, tag="NL")
                dot_segments(
                    NG3.unsqueeze(1).to_broadcast([P, NUM_SENSE, NEG, D]),
                    SEN3.unsqueeze(2).to_broadcast([P, NUM_SENSE, NEG, D]),
                    TMP[:, 0:SN * D].rearrange("p (s n d) -> p s n d",
                                               s=NUM_SENSE, n=NEG),
                    NL[:], SN)
                EN = sp.tile([P, SN], F32, tag="EN")
                nc.scalar.activation(out=EN[:], in_=NL[:], func=AF.Exp,
                                     scale=-1.0)
                nc.vector.tensor_scalar_add(out=EN[:], in0=EN[:], scalar1=1.0)
                NP = sp.tile([P, SN], F32, tag="NP")
                nc.vector.reciprocal(NP[:], EN[:])

                Wn = sp.tile([P, SN], F32, tag="Wn")
                for s in range(NUM_SENSE):
                    nc.vector.tensor_scalar_mul(
                        out=Wn[:, s * NEG:(s + 1) * NEG],
                        in0=NP[:, s * NEG:(s + 1) * NEG],
                        scalar1=ALS[:, s:s + 1])
                tt(Wn[:, 0:NEG], Wn[:, 0:NEG], Wn[:, NEG:2 * NEG])
                tt(Wn[:, 0:NEG], Wn[:, 0:NEG], Wn[:, 2 * NEG:3 * NEG])
                # 1 - x, clamp, ln
                nc.vector.tensor_scalar(
                    out=Wn[:, NEG:2 * NEG], in0=Wn[:, 0:NEG],
                    scalar1=-1.0, scalar2=1.0, op0=OP.mult, op1=OP.add)
                nc.vector.tensor_scalar_max(
                    out=Wn[:, NEG:2 * NEG], in0=Wn[:, NEG:2 * NEG],
                    scalar1=TINY)
                WLn = sp.tile([P, NEG], F32, tag="WLn")
                nc.scalar.activation(out=WLn[:], in_=Wn[:, NEG:2 * NEG],
                                     func=AF.Ln,
                                     accum_out=acc[:, 2 * t + 1:2 * t + 2])

            total = pp.tile([P, 1], F32)
            nc.vector.tensor_reduce(out=total[:], in_=acc[:], axis=AX.X,
                                    op=OP.add)
            ps = psp.tile([1, 1], F32)
            nc.tensor.matmul(out=ps[:], lhsT=total[:], rhs=ones[:],
                             start=True, stop=True)
            res = pp.tile([1, 1], F32)
            nc.scalar.copy(res[:], ps[:])
            nc.sync.dma_start(out=out_d[:, :], in_=res[:])

    nc.compile()
    _CACHE[key] = nc
    return nc


def _pack_idx_cols(vals):
    """flat int array (len % 16 == 0) -> [128, len/16] int16; value i at
    [i%16, i//16], replicated across the 8 partition groups."""
    n = len(vals)
    v = np.asarray(vals, dtype=np.int16).reshape(n // 16, 16).T  # [16, n/16]
    return np.tile(v, (8, 1))


def _prep_inputs(word_ids, context_ids, neg_ids,
                 emb_weight, global_emb_weight, disamb_weight):
    import ml_dtypes
    BF = ml_dtypes.bfloat16
    word_ids = np.asarray(word_ids).astype(np.int64).reshape(BATCH)
    context_ids = np.asarray(context_ids).astype(np.int64).reshape(BATCH, CTX)
    neg_ids = np.asarray(neg_ids).astype(np.int64).reshape(BATCH, NEG)

    packed = np.zeros((VOCAB, ROW), dtype=BF)
    packed[:, 0:D] = np.asarray(
        global_emb_weight, dtype=np.float32).astype(BF).reshape(VOCAB, D)
    packed[:, EMB_OFF:DIS_OFF] = np.asarray(
        emb_weight, dtype=np.float32).astype(BF).reshape(VOCAB, NUM_SENSE * D)
    packed[:, DIS_OFF:SEL_W] = np.asarray(
        disamb_weight, dtype=np.float32).astype(BF).reshape(VOCAB,
                                                            NUM_SENSE * D)
    negtab = np.zeros((VOCAB, NEG_W), dtype=BF)
    negtab[:, 0:D] = packed[:, 0:D]
    negtab = negtab.reshape(VOCAB // 2, NEG_ELEM)
    packed = packed.reshape(VOCAB // 2, WIN)

    in_maps = []
    for core in range(N_CORES):
        sl = slice(core * PER_CORE, (core + 1) * PER_CORE)
        ctx_c = context_ids[sl]          # [512, 10]
        neg_c = neg_ids[sl]              # [512, 5]
        wrd_c = word_ids[sl]             # [512]
        idx_blocks = []
        par_blocks = []
        for t in range(TILES):
            ts = slice(t * P, (t + 1) * P)
            ctx_t, neg_t, wrd_t = ctx_c[ts], neg_c[ts], wrd_c[ts]
            # ctx halves: flat[c_local*128 + p] = id >> 1
            idx_blocks.append(_pack_idx_cols(
                (ctx_t[:, 0:5].T >> 1).reshape(-1)))
            idx_blocks.append(_pack_idx_cols(
                (ctx_t[:, 5:10].T >> 1).reshape(-1)))
            idx_blocks.append(_pack_idx_cols((neg_t.T >> 1).reshape(-1)))
            idx_blocks.append(_pack_idx_cols(wrd_t >> 1))
            par_t = np.zeros((P, PAR_COLS), dtype=np.float32)
            par_t[:, 0:CTX] = ctx_t & 1
            par_t[:, CTX:CTX + NEG] = neg_t & 1
            par_t[:, CTX + NEG] = wrd_t & 1
            h = PAR_COLS // 2
            par_t[:, h:h + CTX + NEG + 1] = 1.0 - par_t[:, 0:CTX + NEG + 1]
            par_blocks.append(par_t)
        in_maps.append({
            "packed": packed,
            "negtab": negtab,
            "idx": np.ascontiguousarray(np.concatenate(idx_blocks, axis=1)),
            "par": np.ascontiguousarray(np.concatenate(par_blocks, axis=1)),
        })
    return in_maps


def kernel(word_ids, context_ids, context_masks, neg_ids,
           emb_weight, global_emb_weight, disamb_weight):
    from concourse import bass_utils
    nc = _build_bass()
    in_maps = _prep_inputs(word_ids, context_ids, neg_ids,
                           emb_weight, global_emb_weight, disamb_weight)
    res = bass_utils.run_bass_kernel_spmd(nc, in_maps,
                                          core_ids=list(range(N_CORES)))
    total = 0.0
    for r in res.results:
        total += float(np.asarray(r["out"]).reshape(-1)[0])
    loss = -total / float(BATCH * CTX)
    return np.array(loss, dtype=np.float32)


# revision 13
# speedup vs baseline: 1.8615x; 1.8615x over previous
"""Multi-sense skip-gram (MSSG) loss kernel for Trainium2 — v2.

Data-parallel over batch across 8 NeuronCores; tables replicated in bf16.

v2 replaces v1's 64 serialized one-index-per-partition indirect DMAs per
core with 16 vectorized SWDGE dma_gather calls (InstDMAGatherAnt, mlp Q7
library): each call gathers hundreds of rows in one instruction, with
descriptors generated at ~0.34ns each and sprayed across all 16 DMA
engines.

dma_gather indices are int16 (sign-extended by the ucode), so vocab ids
up to 49999 cannot be used directly. Pairing trick: the packed table
([50000, 2176] bf16 rows: [glob 300 | emb 900 | dis 900 | pad]) is viewed
as [25000, 4352] two-row windows; idx = id>>1 <= 24999 fits int16. The
correct half is selected on-device: sel = A*(1-par) + B*par, exact since
par is 0.0/1.0, with both parity and complement supplied as host-packed
[P,1] f32 columns. The A-half multiply runs as an ACT-engine Copy-with-
scale (ACT is otherwise idle), the B-half fused multiply-add on DVE.
Costs 2x gather bytes vs. the ideal but is ~100x less per-descriptor
overhead than the v1 path. Negative samples only need the global
embedding, so they gather 1280B pair windows from a separate compact
[25000, 640] glob table instead of full 8704B row pairs.

Compute (per 128-element tile, one element per SBUF partition) is
unchanged from v1: DVE does batched dots/weighted sums along the free
dim, ACT does exp/ln, final partition-reduce via ones-vector matmul on
PE. Host sums the 8 per-core partials.
"""

import numpy as np

NUM_SENSE = 3
EMB_DIM = 300
VOCAB = 50000
BATCH = 4096
CTX = 10
NEG = 5
N_CORES = 8
P = 128
PER_CORE = BATCH // N_CORES  # 512
TILES = PER_CORE // P        # 4
D = EMB_DIM
CS = CTX * NUM_SENSE         # 30
SN = NUM_SENSE * NEG         # 15

ROW = 2176                   # bf16 elems per SELC row stride
ROW8 = 2304                  # fp8 bytes per packed row (9*256)
WIN8 = 2 * ROW8              # fp8 pair window (4608B)
NEG_W8 = 512                 # neg table: per-row glob slot (fp8, 2*256)
NEG_ELEM8 = 2 * NEG_W8       # 1024B pair window
EMB_OFF = D                  # 300
DIS_OFF = D + NUM_SENSE * D  # 1200
SEL_W = DIS_OFF + NUM_SENSE * D  # 2100 used elems per row

# idx columns per tile: ctxA 40 | ctxB 40 | neg 40 | word 8 = 128
IDX_COLS = 128
PAR_COLS = 32                # [par: ctx 10|neg 5|word 1] + [1-par x16]

_CACHE = {}


def _build_bass(tiles=TILES, passes=1):
    key = ("v2", tiles, passes)
    if key in _CACHE:
        return _CACHE[key]

    import concourse.bass as bass
    import concourse.bacc as bacc
    import concourse.tile as tile
    from concourse import mybir
    from concourse.library_config import mlp

    F32 = mybir.dt.float32
    F8 = mybir.dt.float8e4
    BF16 = mybir.dt.bfloat16
    I16 = mybir.dt.int16
    AX = mybir.AxisListType
    OP = mybir.AluOpType
    AF = mybir.ActivationFunctionType
    TINY = float(np.finfo(np.float32).tiny)

    nc = bacc.Bacc("TRN2", target_bir_lowering=False, debug=False)

    packed = nc.dram_tensor("packed", [VOCAB // 2, WIN8], F8,
                            kind="ExternalInput")
    negtab = nc.dram_tensor("negtab", [VOCAB // 2, NEG_ELEM8], F8,
                            kind="ExternalInput")
    idx = nc.dram_tensor("idx", [P, tiles * IDX_COLS], I16,
                         kind="ExternalInput")
    par = nc.dram_tensor("par", [P, tiles * PAR_COLS], F32,
                         kind="ExternalInput")
    out_d = nc.dram_tensor("out", [1, 1], F32, kind="ExternalOutput")

    def tt(out, a, b, op=OP.add):
        nc.vector.tensor_tensor(out=out, in0=a, in1=b, op=op)

    with tile.TileContext(nc) as tc:
        with (
            tc.tile_pool(name="gw", bufs=2) as gw,
            tc.tile_pool(name="sel", bufs=1) as slp,
            tc.tile_pool(name="tmpp", bufs=1) as tp,
            tc.tile_pool(name="small", bufs=2) as sp,
            tc.tile_pool(name="persist", bufs=1) as pp,
            tc.tile_pool(name="psum", bufs=1, space="PSUM") as psp,
        ):
            nc.gpsimd.load_library(mlp)

            acc = pp.tile([P, 2 * TILES], F32)
            ones = pp.tile([P, 1], F32)
            nc.vector.memset(ones[:], 1.0)
            IX = pp.tile([P, tiles * IDX_COLS], I16)
            PAR = pp.tile([P, tiles * PAR_COLS], F32)
            nc.sync.dma_start(out=IX[:], in_=idx[:, :])
            nc.sync.dma_start(out=PAR[:], in_=par[:, :])

            SELC = slp.tile([P, CTX * ROW], BF16)   # selected ctx rows
            SELN = slp.tile([P, NEG * 320], BF16)   # selected neg glob
            SELW = slp.tile([P, 2 * NUM_SENSE * D], BF16)  # word emb|dis

            for t_iter in range(tiles * passes):
                t = t_iter % tiles
                ib = t * IDX_COLS
                pb = t * PAR_COLS

                def select(dst, a8, b8, par_col, width):
                    # dst(bf16) = a8*(1-par)/1024 + b8*par/1024
                    # both fp8 dequant-scales run on the ACT engine; DVE
                    # only does one bf16 add (fp8 reads would drop DVE to
                    # 1x mode)
                    omp_col = pb + PAR_COLS // 2 + (par_col - pb)
                    scr = sp.tile([P, SEL_W], BF16, tag="scr")
                    nc.scalar.activation(
                        out=dst, in_=a8, func=AF.Copy,
                        scale=PAR[:, omp_col:omp_col + 1])
                    nc.scalar.activation(
                        out=scr[:, 0:width], in_=b8, func=AF.Copy,
                        scale=PAR[:, par_col:par_col + 1])
                    tt(dst, dst, scr[:, 0:width], OP.add)

                # ctx: 2 gathers x 640 windows
                for h in range(2):
                    W = gw.tile([P, 5 * WIN8], F8, tag="wc")
                    W3 = W[:].rearrange("p (c w) -> p c w", w=WIN8)
                    nc.gpsimd.dma_gather(
                        W3, packed[:, :],
                        IX[:, ib + h * 40:ib + (h + 1) * 40],
                        640, 640, WIN8)
                    for c in range(5):
                        cc = h * 5 + c
                        select(SELC[:, cc * ROW:cc * ROW + SEL_W],
                               W3[:, c, 0:SEL_W],
                               W3[:, c, ROW8:ROW8 + SEL_W],
                               pb + cc, SEL_W)

                # neg: 640 pair windows from the compact glob table
                WN = gw.tile([P, 5 * NEG_ELEM8], F8, tag="wn")
                WN3 = WN[:].rearrange("p (c w) -> p c w", w=NEG_ELEM8)
                nc.gpsimd.dma_gather(
                    WN3, negtab[:, :],
                    IX[:, ib + 80:ib + 120],
                    640, 640, NEG_ELEM8)
                for n in range(5):
                    select(SELN[:, n * 320:n * 320 + 300],
                           WN3[:, n, 0:D], WN3[:, n, NEG_W8:NEG_W8 + D],
                           pb + 10 + n, D)

                # word: 128 windows; only emb|dis sections selected
                WW = gw.tile([P, WIN8], F8, tag="ww")
                WW3 = WW[:].rearrange("p (c w) -> p c w", w=WIN8)
                nc.gpsimd.dma_gather(
                    WW3, packed[:, :],
                    IX[:, ib + 120:ib + 128],
                    128, 128, WIN8)
                select(SELW[:],
                       WW3[:, 0, EMB_OFF:SEL_W],
                       WW3[:, 0, ROW8 + EMB_OFF:ROW8 + SEL_W],
                       pb + 15, SEL_W - EMB_OFF)

                # ---- views over selected rows ----
                SEL3 = SELC[:].rearrange("p (c x) -> p c x", x=ROW)
                CT3 = SEL3[:, :, 0:D]                                  # [P,10,300]
                AS4 = SEL3[:, :, EMB_OFF:DIS_OFF].rearrange(
                    "p c (s d) -> p c s d", d=D)                       # [P,10,3,300]
                AD4 = SEL3[:, :, DIS_OFF:SEL_W].rearrange(
                    "p c (s d) -> p c s d", d=D)
                SEN3 = SELW[:, 0:NUM_SENSE * D].rearrange(
                    "p (s d) -> p s d", d=D)                           # [P,3,300]
                DIS3 = SELW[:, NUM_SENSE * D:2 * NUM_SENSE * D].rearrange(
                    "p (s d) -> p s d", d=D)
                NG3 = SELN[:].rearrange("p (n x) -> p n x", x=320)[:, :, 0:D]

                def as_seg(k):
                    c, s = k // NUM_SENSE, k % NUM_SENSE
                    return AS4[:, c, s, :]

                TMP = tp.tile([P, CS * D], BF16)

                def seg3(ap):
                    return ap.rearrange("p (c d) -> p c d", d=D)

                def dot_segments(a_full, b_full, out_full, zout, nseg):
                    """zout[:, k] = sum_d a_seg(k) * b_seg(k) (dots of 300-vecs)
                    via elementwise mult into TMP + fold tree + short reduce
                    (DVE tensor_reduce runs at ~1 elem/cycle vs 2x for TT, so
                    folds first, reduce only the last 38)."""
                    tt(out_full, a_full, b_full, OP.mult)
                    pr = seg3(TMP[:])[:, 0:nseg, :]
                    tt(pr[:, :, 0:150], pr[:, :, 0:150], pr[:, :, 150:300])
                    tt(pr[:, :, 0:74], pr[:, :, 0:74], pr[:, :, 76:150])
                    tt(pr[:, :, 0:38], pr[:, :, 0:38], pr[:, :, 38:76])
                    nc.vector.tensor_reduce(
                        out=zout, in_=pr[:, :, 0:38], axis=AX.X, op=OP.add)

                # ---- ctx1 = sum_c CT (mean deferred via exp-scale) ----
                # fold tree through TMP slices (TMP is free at this point)
                c1a = TMP[:, 0:5 * D]
                c1b = TMP[:, 5 * D:7 * D]
                ctx1 = sp.tile([P, D], BF16, tag="ctx1")
                tt(seg3(c1a), CT3[:, 0:5, :], CT3[:, 5:10, :])
                tt(c1b, c1a[:, 0:2 * D], c1a[:, 2 * D:4 * D])
                tt(c1b[:, 0:D], c1b[:, 0:D], c1b[:, D:2 * D])
                tt(ctx1[:], c1b[:, 0:D], c1a[:, 4 * D:5 * D])

                def disamb_step(ctx_vec, ctx_out):
                    z = sp.tile([P, CS], F32, tag="z")
                    dot_segments(
                        AD4,
                        ctx_vec[:].unsqueeze(1).unsqueeze(1)
                                  .to_broadcast([P, CTX, NUM_SENSE, D]),
                        TMP[:].rearrange("p (c s d) -> p c s d",
                                         s=NUM_SENSE, d=D),
                        z[:], CS)
                    E = sp.tile([P, CS], F32, tag="E")
                    nc.scalar.activation(out=E[:], in_=z[:], func=AF.Exp,
                                         scale=1.0 / CTX)
                    S = sp.tile([P, CTX], F32, tag="S")
                    nc.vector.tensor_reduce(
                        out=S[:], in_=E[:].rearrange("p (c s) -> p c s",
                                                     s=NUM_SENSE),
                        axis=AX.X, op=OP.add)
                    R = sp.tile([P, CTX], F32, tag="R")
                    nc.vector.reciprocal(R[:], S[:])
                    AL = sp.tile([P, CS], F32, tag="AL")
                    tt(AL[:].rearrange("p (c s) -> p c s", s=NUM_SENSE),
                       E[:].rearrange("p (c s) -> p c s", s=NUM_SENSE),
                       R[:].unsqueeze(2).to_broadcast([P, CTX, NUM_SENSE]),
                       OP.mult)
                    T3 = seg3(TMP[:])
                    for cs in range(CS):
                        nc.vector.tensor_scalar_mul(
                            out=T3[:, cs, :], in0=as_seg(cs),
                            scalar1=AL[:, cs:cs + 1])
                    tt(T3[:, 0:15, :], T3[:, 0:15, :], T3[:, 15:30, :])
                    tt(T3[:, 0:7, :], T3[:, 0:7, :], T3[:, 7:14, :])
                    tt(T3[:, 0:3, :], T3[:, 0:3, :], T3[:, 3:6, :])
                    tt(T3[:, 0:1, :], T3[:, 0:1, :], T3[:, 1:2, :])
                    tt(T3[:, 0:1, :], T3[:, 0:1, :], T3[:, 14:15, :])
                    tt(T3[:, 2:3, :], T3[:, 2:3, :], T3[:, 6:7, :])
                    tt(ctx_out[:].unsqueeze(1), T3[:, 0:1, :], T3[:, 2:3, :])

                ctx2 = sp.tile([P, D], BF16, tag="ctx2")
                ctx3 = sp.tile([P, D], BF16, tag="ctx3")
                disamb_step(ctx1, ctx2)
                disamb_step(ctx2, ctx3)

                # ---- alpha = softmax_s(DIS . ctx3/C) ----
                av = sp.tile([P, NUM_SENSE], F32, tag="av")
                dot_segments(
                    DIS3,
                    ctx3[:].unsqueeze(1).to_broadcast([P, NUM_SENSE, D]),
                    seg3(TMP[:])[:, 0:NUM_SENSE, :], av[:], NUM_SENSE)
                Ea = sp.tile([P, NUM_SENSE], F32, tag="Ea")
                nc.scalar.activation(out=Ea[:], in_=av[:], func=AF.Exp,
                                     scale=1.0 / CTX)
                Sa = sp.tile([P, 1], F32, tag="Sa")
                nc.vector.tensor_reduce(out=Sa[:], in_=Ea[:], axis=AX.X,
                                        op=OP.add)
                Ra = sp.tile([P, 1], F32, tag="Ra")
                nc.vector.reciprocal(Ra[:], Sa[:])
                ALS = sp.tile([P, NUM_SENSE], F32, tag="ALS")
                nc.vector.tensor_scalar_mul(out=ALS[:], in0=Ea[:],
                                            scalar1=Ra[:, 0:1])

                # ---- pos: sigmoid(SEN . CT) over (s,c) ----
                PL = sp.tile([P, CS], F32, tag="PL")
                dot_segments(
                    CT3.unsqueeze(1).to_broadcast([P, NUM_SENSE, CTX, D]),
                    SEN3.unsqueeze(2).to_broadcast([P, NUM_SENSE, CTX, D]),
                    TMP[:].rearrange("p (s c d) -> p s c d",
                                     s=NUM_SENSE, c=CTX),
                    PL[:], CS)
                EP = sp.tile([P, CS], F32, tag="EP")
                nc.scalar.activation(out=EP[:], in_=PL[:], func=AF.Exp,
                                     scale=-1.0)
                nc.vector.tensor_scalar_add(out=EP[:], in0=EP[:], scalar1=1.0)
                PP = sp.tile([P, CS], F32, tag="PP")
                nc.vector.reciprocal(PP[:], EP[:])

                # pos_term[c] = sum_s alpha_s * PP[s,c]
                W_ = sp.tile([P, CS], F32, tag="W")
                for s in range(NUM_SENSE):
                    nc.vector.tensor_scalar_mul(
                        out=W_[:, s * CTX:(s + 1) * CTX],
                        in0=PP[:, s * CTX:(s + 1) * CTX],
                        scalar1=ALS[:, s:s + 1])
                tt(W_[:, 0:CTX], W_[:, 0:CTX], W_[:, CTX:2 * CTX])
                tt(W_[:, 0:CTX], W_[:, 0:CTX], W_[:, 2 * CTX:3 * CTX])
                nc.vector.tensor_scalar_max(out=W_[:, 0:CTX], in0=W_[:, 0:CTX],
                                            scalar1=TINY)
                WL = sp.tile([P, CTX], F32, tag="WL")
                nc.scalar.activation(out=WL[:], in_=W_[:, 0:CTX], func=AF.Ln,
                                     accum_out=acc[:, 2 * t:2 * t + 1])

                # ---- neg: sigmoid(SEN . NG) over (s,n) ----
                NL = sp.tile([P, SN], F32, tag="NL")
                dot_segments(
                    NG3.unsqueeze(1).to_broadcast([P, NUM_SENSE, NEG, D]),
                    SEN3.unsqueeze(2).to_broadcast([P, NUM_SENSE, NEG, D]),
                    TMP[:, 0:SN * D].rearrange("p (s n d) -> p s n d",
                                               s=NUM_SENSE, n=NEG),
                    NL[:], SN)
                EN = sp.tile([P, SN], F32, tag="EN")
                nc.scalar.activation(out=EN[:], in_=NL[:], func=AF.Exp,
                                     scale=-1.0)
                nc.vector.tensor_scalar_add(out=EN[:], in0=EN[:], scalar1=1.0)
                NP = sp.tile([P, SN], F32, tag="NP")
                nc.vector.reciprocal(NP[:], EN[:])

                Wn = sp.tile([P, SN], F32, tag="Wn")
                for s in range(NUM_SENSE):
                    nc.vector.tensor_scalar_mul(
                        out=Wn[:, s * NEG:(s + 1) * NEG],
                        in0=NP[:, s * NEG:(s + 1) * NEG],
                        scalar1=ALS[:, s:s + 1])
                tt(Wn[:, 0:NEG], Wn[:, 0:NEG], Wn[:, NEG:2 * NEG])
                tt(Wn[:, 0:NEG], Wn[:, 0:NEG], Wn[:, 2 * NEG:3 * NEG])
                # 1 - x, clamp, ln
                nc.vector.tensor_scalar(
                    out=Wn[:, NEG:2 * NEG], in0=Wn[:, 0:NEG],
                    scalar1=-1.0, scalar2=1.0, op0=OP.mult, op1=OP.add)
                nc.vector.tensor_scalar_max(
                    out=Wn[:, NEG:2 * NEG], in0=Wn[:, NEG:2 * NEG],
                    scalar1=TINY)
                WLn = sp.tile([P, NEG], F32, tag="WLn")
                nc.scalar.activation(out=WLn[:], in_=Wn[:, NEG:2 * NEG],
                                     func=AF.Ln,
                                     accum_out=acc[:, 2 * t + 1:2 * t + 2])

            total = pp.tile([P, 1], F32)
            nc.vector.tensor_reduce(out=total[:], in_=acc[:], axis=AX.X,
                                    op=OP.add)
            ps = psp.tile([1, 1], F32)
            nc.tensor.matmul(out=ps[:], lhsT=total[:], rhs=ones[:],
                             start=True, stop=True)
            res = pp.tile([1, 1], F32)
            nc.scalar.copy(res[:], ps[:])
            nc.sync.dma_start(out=out_d[:, :], in_=res[:])

    nc.compile()
    _CACHE[key] = nc
    return nc


def _pack_idx_cols(vals):
    """flat int array (len % 16 == 0) -> [128, len/16] int16; value i at
    [i%16, i//16], replicated across the 8 partition groups."""
    n = len(vals)
    v = np.asarray(vals, dtype=np.int16).reshape(n // 16, 16).T  # [16, n/16]
    return np.tile(v, (8, 1))


def _prep_inputs(word_ids, context_ids, neg_ids,
                 emb_weight, global_emb_weight, disamb_weight):
    import ml_dtypes
    BF = ml_dtypes.bfloat16
    word_ids = np.asarray(word_ids).astype(np.int64).reshape(BATCH)
    context_ids = np.asarray(context_ids).astype(np.int64).reshape(BATCH, CTX)
    neg_ids = np.asarray(neg_ids).astype(np.int64).reshape(BATCH, NEG)

    F8 = ml_dtypes.float8_e4m3
    SCALE = 1024.0
    packed = np.zeros((VOCAB, ROW8), dtype=F8)
    packed[:, 0:D] = (np.asarray(
        global_emb_weight, dtype=np.float32).reshape(VOCAB, D)
        * SCALE).astype(F8)
    packed[:, EMB_OFF:DIS_OFF] = (np.asarray(
        emb_weight, dtype=np.float32).reshape(VOCAB, NUM_SENSE * D)
        * SCALE).astype(F8)
    packed[:, DIS_OFF:SEL_W] = (np.asarray(
        disamb_weight, dtype=np.float32).reshape(VOCAB, NUM_SENSE * D)
        * SCALE).astype(F8)
    negtab = np.zeros((VOCAB, NEG_W8), dtype=F8)
    negtab[:, 0:D] = packed[:, 0:D]
    negtab = negtab.reshape(VOCAB // 2, NEG_ELEM8)
    packed = packed.reshape(VOCAB // 2, WIN8)

    in_maps = []
    for core in range(N_CORES):
        sl = slice(core * PER_CORE, (core + 1) * PER_CORE)
        ctx_c = context_ids[sl]          # [512, 10]
        neg_c = neg_ids[sl]              # [512, 5]
        wrd_c = word_ids[sl]             # [512]
        idx_blocks = []
        par_blocks = []
        for t in range(TILES):
            ts = slice(t * P, (t + 1) * P)
            ctx_t, neg_t, wrd_t = ctx_c[ts], neg_c[ts], wrd_c[ts]
            # ctx halves: flat[c_local*128 + p] = id >> 1
            idx_blocks.append(_pack_idx_cols(
                (ctx_t[:, 0:5].T >> 1).reshape(-1)))
            idx_blocks.append(_pack_idx_cols(
                (ctx_t[:, 5:10].T >> 1).reshape(-1)))
            idx_blocks.append(_pack_idx_cols((neg_t.T >> 1).reshape(-1)))
            idx_blocks.append(_pack_idx_cols(wrd_t >> 1))
            par_t = np.zeros((P, PAR_COLS), dtype=np.float32)
            par_t[:, 0:CTX] = ctx_t & 1
            par_t[:, CTX:CTX + NEG] = neg_t & 1
            par_t[:, CTX + NEG] = wrd_t & 1
            h = PAR_COLS // 2
            par_t[:, h:h + CTX + NEG + 1] = 1.0 - par_t[:, 0:CTX + NEG + 1]
            par_blocks.append(par_t / 1024.0)
        in_maps.append({
            "packed": packed,
            "negtab": negtab,
            "idx": np.ascontiguousarray(np.concatenate(idx_blocks, axis=1)),
            "par": np.ascontiguousarray(np.concatenate(par_blocks, axis=1)),
        })
    return in_maps


def kernel(word_ids, context_ids, context_masks, neg_ids,
           emb_weight, global_emb_weight, disamb_weight):
    from concourse import bass_utils
    nc = _build_bass()
    in_maps = _prep_inputs(word_ids, context_ids, neg_ids,
                           emb_weight, global_emb_weight, disamb_weight)
    res = bass_utils.run_bass_kernel_spmd(nc, in_maps,
                                          core_ids=list(range(N_CORES)))
    total = 0.0
    for r in res.results:
        total += float(np.asarray(r["out"]).reshape(-1)[0])
    loss = -total / float(BATCH * CTX)
    return np.array(loss, dtype=np.float32)
